# revision 26
# baseline (speedup 1.0000x reference)
"""Dual attention (DANet-style spatial + channel attention) on 8 Trainium2
NeuronCores.

Sharding: data-parallel over batch B=4, each batch's output positions split in
half across 2 cores -> 8 identical single-core programs, no collectives. The
host permutes each core's x so its OWN m-half occupies columns [0, M): softmax
/ sums over n are permutation-invariant, so k/v/p built in permuted order are
fine, and q / residual / output always use columns [0, M).

Per-core math (x: [512, 4096] f16, m-chunk: 2048 positions):
  spatial:  q=Wq@x[:, :M]+bq; k=Wk@x+bk; E^T[n,m]=k[:,n].q[:,m]; P=exp(E^T)
            vT[n,c]=(Wv@x+bv)^T
            U^T[m,c]=sum_n P[n,m] vT[n,c]   (lhsT = P m-slices -> out has m on
            partitions, so 1/Z is a per-PARTITION scalar: no broadcast chain)
            Z[m]=sum_n P[n,m] via 4 tiny matmuls from the DVE-accumulated S
  channel:  pT[n,d]=(Wd@x+bd)^T; e=pT^T@pT; c_attn=softmax(rowmax(e)-e)
            c2=gamma_c*(c_attn@p)[:, :M]+p[:, :M]
            R^T[m,c]=Wu@c2+bu+x[:, :M]  (bu via an appended ones-row: contract
            65; residual from host-transposed xmT f16)
  out^T[m,c] = U^T*(gamma_s/Z)[m] + R^T   -> DRAM [M, C], host transposes.

Performance structure (272us v0 baseline -> ~219us at full 2.4GHz clock;
the chip's P0 power-state can downclock the PE to 2.0GHz under sustained
load, which inflates any single measurement ~19%):
  - chunk 0 of the spatial attention is FUSED into the vT/pT production
    loop: U(0, nt) consumes vT[nt-2] (two iterations behind, so the DVE
    bias-add and exp semaphores fire long before U's weight loads want
    them), and the PE rolls from projections into attention with zero
    idle -- HAM stays at 2.4GHz for the whole kernel (PE ~98% busy between
    first and last matmul).
  - vT is produced by fp8-e4m3 DoubleRow matmuls (x8/wv8 operand pairs
    [128, 2, free]): half the matmuls of the f16 conv at ~1e-3 extra error.
  - E matmuls: contract is only 64, so q/k are computed DOUBLED ([Wq|Wq]
    conv weights -> identical copies on partitions 0-63/64-127) and even/odd
    n-tiles issue to different PE row-groups via tile_position -- adjacent E
    matmuls execute concurrently (2 per 512-col slot). E issues 2-3 tiles
    ahead of its U group so ACT exp latency is hidden.
  - one permanent PSUM pool set (u:4 + e:3 + aux:1 banks) spans conv /
    fused / chunk phases: no pool-scope handoffs, no PE bubbles between
    phases. Z is 4 tiny matmuls in one bank (one accumulation group).
  - the epilogue is per-partition: U banks release via plain DVE copies (no
    Z dependency; skipped for the last chunk), combine is one
    scalar_tensor_tensor per m-subtile, output DMAs as [M, C] rows
    (2KB/partition lines); channel c2/R^T matmuls interleave into chunk 1.
  - DMA in is 8.9MB as ~10 large descriptors (each stripes 16-ways across
    the DMA engines at ~320GB/s; the ~650ns PER-DESCRIPTOR serial issue on
    the sync queue is what must stay small). The consts image ships
    gamma_s / bd-row / bv-row pre-broadcast across partitions.
"""
import sys

sys.path.insert(0, '/opt/trn_rl_repo')

import numpy as np

import concourse.bass as bass
import concourse.tile as tile
from concourse import bacc, bass_utils, mybir
from concourse.masks import make_identity

# Problem shapes (fixed by the task spec)
B, C, WIDTH, HEIGHT = 4, 512, 64, 64
N = WIDTH * HEIGHT      # 4096 spatial positions
DK = 64                 # attention inner dim (and channel-attn dim)
NCORES = 8
M = N // 2              # 2048 output positions per core
P = 128
KC = C // P             # 4 input-channel chunks
NT = N // P             # 32 key-position tiles
FREE = 512              # matmul moving free dim (one PSUM bank of fp32)
MCH = M // FREE         # 4 m-chunks per core
MS = FREE // P          # 4 m-subtiles (128 rows) per chunk
MT = M // P             # 16 m-subtiles total

F32 = mybir.dt.float32
F16 = mybir.dt.float16
BF16 = mybir.dt.bfloat16
AX = mybir.AxisListType
ALU = mybir.AluOpType
ACTF = mybir.ActivationFunctionType

# byte layout of the packed-constants image (per partition)
OFF_WQ, OFF_WK = 0, 1024     # [128, kc, 128] f16: [Wq|Wq], [Wk|Wk] doubled
OFF_WD = 2048                # [128, kc, 64] f16
OFF_BQ, OFF_BK, OFF_BD, OFF_GC = 2560, 2564, 2568, 2572
OFF_GS = 2576                # [128, 1] f32, replicated on all partitions
OFF_P2 = 2592                # consts DMA split: everything below lands first
OFF_WUB = 2592               # [65, 512] f16: rows 0-63 Wu^T, row 64 = bu
OFF_BDB = 3616               # [128, 64] f32, bd row replicated on all partitions
OFF_BVB = 3872               # [128, 512] f32, bv row replicated on all partitions
PKB = 5920


def _build_program(tc, io):
    nc = tc.nc
    x_d, xmT_d, out_d = io['x'], io['xmT'], io['out']

    const_cm = tc.tile_pool(name='const', bufs=1)
    const = const_cm.__enter__()

    # ---- persistent SBUF tensors ----
    pk_sb = const.tile([P, PKB], mybir.dt.uint8)
    # conv weights + ACT biases first: the conv pipeline waits only on
    # this piece; wub/broadcast rows land behind x chunk 0
    nc.sync.dma_start(pk_sb[:, 0:OFF_P2], io['consts'][:, 0:OFF_P2])
    wq_sb = pk_sb[:, OFF_WQ:OFF_WQ + 1024].bitcast(F16).rearrange(
        "p (kc d) -> p kc d", kc=KC)
    wk_sb = pk_sb[:, OFF_WK:OFF_WK + 1024].bitcast(F16).rearrange(
        "p (kc d) -> p kc d", kc=KC)
    wd_sb = pk_sb[:, OFF_WD:OFF_WD + 512].bitcast(F16).rearrange(
        "p (kc d) -> p kc d", kc=KC)
    wub_sb = pk_sb[0:DK + 1, OFF_WUB:OFF_WUB + 1024].bitcast(F16)
    bq_sb = pk_sb[:, OFF_BQ:OFF_BQ + 4].bitcast(F32)
    bk_sb = pk_sb[:, OFF_BK:OFF_BK + 4].bitcast(F32)
    bd_sb = pk_sb[0:DK, OFF_BD:OFF_BD + 4].bitcast(F32)
    gc_sb = pk_sb[0:DK, OFF_GC:OFF_GC + 4].bitcast(F32)
    gs_sb = pk_sb[:, OFF_GS:OFF_GS + 4].bitcast(F32)
    bdb_sb = pk_sb[:, OFF_BDB:OFF_BDB + 256].bitcast(F32)
    bvb_sb = pk_sb[:, OFF_BVB:OFF_BVB + 2048].bitcast(F32)

    ones_colb = const.tile([P, 1], BF16)   # rhs for the tiny Z matmuls
    nc.vector.memset(ones_colb[:], 1.0)
    ident16 = const.tile([DK, DK], F16)    # for the tiny c_attn transpose
    make_identity(nc, ident16[:])

    k_sb = const.tile([P, N], F16)   # keys [d, n], rows 64-127 = copy
    q_sb = const.tile([P, M], F16)   # queries,   rows 64-127 = copy
    pc_sb = const.tile([DK, M], F16)       # channel proj on the m-slice
    c2b_sb = const.tile([DK + 1, M], F16)  # c2 rows 0-63, row 64 = ones
    pT_sb = const.tile([P, NT, DK], F16)   # channel proj transposed [n, nt, d]
    vT_sb = const.tile([P, NT, C], BF16)   # values transposed, [n, nt, c]
    catT_sb = const.tile([DK, DK], F16)    # c_attn^T for the c2 matmuls
    xmT_sb = const.tile([P, MT, C], F16)   # residual x^T on the m-slice
    r_sb = const.tile([P, MT, C], F32)     # R^T = channel-out + residual

    nc.vector.memset(c2b_sb[DK:DK + 1, :], 1.0)

    wv8_sb = const.tile([P, KC, C], mybir.dt.float8e4)  # fp8 Wv^T for DR

    out_r = out_d.rearrange("(mt p) c -> p mt c", p=P)

    upool_cm = tc.tile_pool(name='upool', bufs=MS, space='PSUM')
    upool = upool_cm.__enter__()
    epool_cm = tc.tile_pool(name='epool', bufs=3, space='PSUM')
    aux_cm = tc.tile_pool(name='aux', bufs=1, space='PSUM')
    epool = aux = None  # entered after the conv pool closes
    ptp_cm = tc.tile_pool(name='pt', bufs=6)
    ptp = ptp_cm.__enter__()
    ssb_cm = tc.tile_pool(name='ssb', bufs=3)
    ssb = ssb_cm.__enter__()
    otp_cm = tc.tile_pool(name='ot', bufs=10)
    otp = otp_cm.__enter__()

    u_ps = {}     # mc -> list of 4 PSUM tiles
    s_sb = {}     # mc -> S accumulator [128, 512] bf16
    p_tl = {}     # nt -> exp tile of the CURRENT chunk
    o1_t = {}     # mc -> 4 SBUF copies of U^T awaiting the combine
    zrg = {}      # mc -> gamma_s/Z [128, 4] f32

    def emit_e(mc, nt, epool):
        # nt parity picks the PE row-group: even tiles compute on array rows
        # 0-63, odd on 64-127, so back-to-back E matmuls run CONCURRENTLY
        # (k/q carry identical copies on partitions 64-127 for this).
        msl = slice(mc * FREE, (mc + 1) * FREE)
        nsl = slice(nt * P, (nt + 1) * P)
        h = (nt & 1) * DK
        e_t = epool.tile([P, FREE], F32, tag='et')
        nc.tensor.matmul(e_t[:], lhsT=k_sb[h:h + DK, nsl],
                         rhs=q_sb[h:h + DK, msl],
                         start=True, stop=True, tile_position=(h, 0))
        p_t = ptp.tile([P, FREE], BF16, tag='p')
        nc.scalar.activation(p_t[:], e_t[:], ACTF.Exp)
        p_tl[nt] = p_t

    def emit_u(mc, nt):
        p_t = p_tl.pop(nt)
        if nt == 0:
            nc.vector.tensor_copy(s_sb[mc][:], p_t[:])
        else:
            nc.vector.tensor_add(s_sb[mc][:], in0=s_sb[mc][:], in1=p_t[:])
        for ms in range(MS):
            nc.tensor.matmul(u_ps[mc][ms][:],
                             lhsT=p_t[:, ms * P:(ms + 1) * P],
                             rhs=vT_sb[:, nt],
                             start=(nt == 0), stop=(nt == NT - 1))

    def emit_release(mc, aux):
        # Free chunk mc's U banks (plain copies, no Z dep) and compute
        # gamma_s/Z [128, 4] -- all per-partition.
        if mc not in o1_t:
            o1_t[mc] = []
            for ms in range(MS):
                o1 = otp.tile([P, FREE], F32, tag='o')
                nc.vector.tensor_copy(o1[:], u_ps[mc][ms][:])
                o1_t[mc].append(o1)
        z_ps = aux.tile([P, MS], F32, tag='aux')
        for ms in range(MS):
            nc.tensor.matmul(z_ps[:, ms:ms + 1],
                             lhsT=s_sb[mc][:, ms * P:(ms + 1) * P],
                             rhs=ones_colb[:],
                             start=(ms == 0), stop=(ms == MS - 1))
        zr_sb = ssb.tile([P, MS], F32, tag='zr')
        nc.vector.reciprocal(zr_sb[:], z_ps[:])
        zg = ssb.tile([P, MS], F32, tag='zrg')
        nc.vector.tensor_scalar_mul(zg[:], in0=zr_sb[:], scalar1=gs_sb[:])
        zrg[mc] = zg

    def emit_combine(mc, ms, eng=None):
        # out = U^T * (gamma_s/Z) + R^T, then store one [128, 512] tile.
        # All inputs are SBUF, so this can run on gpsimd when the DVE is
        # busy (chunk 1 carries the channel-attention DVE work).
        o2 = otp.tile([P, FREE], F32, tag='o')
        (eng or nc.vector).scalar_tensor_tensor(
            out=o2[:], in0=o1_t[mc][ms][:], scalar=zrg[mc][:, ms:ms + 1],
            in1=r_sb[:, mc * MS + ms], op0=ALU.mult, op1=ALU.add)
        nc.sync.dma_start(out_r[:, mc * MS + ms], o2[:])

    def channel_tail(step, aux):
        # c2 = gamma_c * (c_attn @ p)[:, :M] + pc  (4 matmuls), then
        # R^T[mt] = (c2 | ones)^T @ (Wu^T | bu) + xmT  (16 matmuls),
        # interleaved into chunk 1 one step per nt iteration.
        if step < MCH:
            j = step
            sl = slice(j * FREE, (j + 1) * FREE)
            co_ps = aux.tile([DK, FREE], F32, tag='aux')
            nc.tensor.matmul(co_ps[:], lhsT=catT_sb[:], rhs=pc_sb[:, sl],
                             start=True, stop=True)
            nc.vector.scalar_tensor_tensor(
                out=c2b_sb[0:DK, sl], in0=co_ps[:], scalar=gc_sb[:],
                in1=pc_sb[:, sl], op0=ALU.mult, op1=ALU.add)
        else:
            mt = step - MCH
            rw_ps = aux.tile([P, C], F32, tag='aux')
            nc.tensor.matmul(rw_ps[:], lhsT=c2b_sb[:, mt * P:(mt + 1) * P],
                             rhs=wub_sb[:], start=True, stop=True)
            nc.vector.tensor_add(r_sb[:, mt], in0=rw_ps[:], in1=xmT_sb[:, mt])

    # ================= phase 1 + fused chunk 0 =================
    with tc.tile_pool(name='xp', bufs=1) as xp:
        x_sb = xp.tile([P, KC, N], F16)
        x8_sb = xp.tile([P, KC, N], mybir.dt.float8e4)
        x_r = x_d.rearrange("(kc p) n -> p kc n", p=P)
        x8_r = io['x8'].rearrange("(kc p) n -> p kc n", p=P)
        # Every dma_start is striped 16-ways across the DMA engines
        # (~320GB/s data-plane); the cost that matters is the ~650ns SERIAL
        # descriptor issue on the emitting engine queue. So: few descriptors,
        # the first x chunk first (convs start on it), side inputs issued
        # from the otherwise-idle gpsimd queue.
        for kc in range(KC):   # chunk 0 lands per-kc: conv 0 starts sooner
            nc.sync.dma_start(x_sb[:, kc, 0:FREE], x_r[:, kc, 0:FREE])
        # rest of the consts (ACT biases etc.) right behind chunk 0: the
        # first conv's bias-add needs it ~1.5us after the first matmul
        nc.sync.dma_start(pk_sb[:, OFF_P2:PKB], io['consts'][:, OFF_P2:PKB])
        nc.sync.dma_start(x_sb[:, :, FREE:2 * FREE], x_r[:, :, FREE:2 * FREE])
        nc.sync.dma_start(x_sb[:, :, 2 * FREE:4 * FREE],
                          x_r[:, :, 2 * FREE:4 * FREE])
        nc.sync.dma_start(x_sb[:, :, 4 * FREE:6 * FREE],
                          x_r[:, :, 4 * FREE:6 * FREE])
        nc.sync.dma_start(x_sb[:, :, 6 * FREE:N], x_r[:, :, 6 * FREE:N])
        nc.sync.dma_start(wv8_sb[:],
                          io['wv8'].rearrange("(kc p) c -> p kc c", p=P))
        nc.sync.dma_start(x8_sb[:, :, 0:N // 2], x8_r[:, :, 0:N // 2])
        nc.sync.dma_start(x8_sb[:, :, N // 2:N], x8_r[:, :, N // 2:N])
        nc.sync.dma_start(xmT_sb[:],
                          xmT_d.rearrange("(mt p) c -> p mt c", p=P))

        # conv projections, consuming x chunks as they land:
        # q/pc over the own half (cols 0..M), k over the full N
        with tc.tile_pool(name='ps0', bufs=2, space='PSUM') as ps0:
            for j in range(8):
                sl = slice(j * FREE, (j + 1) * FREE)
                if j < MCH:
                    pq = ps0.tile([P, FREE], F32, tag='pq')
                    for kc in range(KC):
                        nc.tensor.matmul(pq[:], lhsT=wq_sb[:, kc],
                                         rhs=x_sb[:, kc, sl],
                                         start=(kc == 0), stop=(kc == KC - 1))
                    nc.scalar.activation(q_sb[:, sl], pq[:], ACTF.Identity,
                                         bias=bq_sb[:])
                    ppc = ps0.tile([DK, FREE], F32, tag='pq')
                    for kc in range(KC):
                        nc.tensor.matmul(ppc[:], lhsT=wd_sb[:, kc],
                                         rhs=x_sb[:, kc, sl],
                                         start=(kc == 0), stop=(kc == KC - 1))
                    nc.scalar.activation(pc_sb[:, sl], ppc[:], ACTF.Identity,
                                         bias=bd_sb[:])
                pk = ps0.tile([P, FREE], F32, tag='pq')
                for kc in range(KC):
                    nc.tensor.matmul(pk[:], lhsT=wk_sb[:, kc],
                                     rhs=x_sb[:, kc, sl],
                                     start=(kc == 0), stop=(kc == KC - 1))
                nc.scalar.activation(k_sb[:, sl], pk[:], ACTF.Identity,
                                     bias=bk_sb[:])
        epool = epool_cm.__enter__()
        aux = aux_cm.__enter__()

        # fused loop: produce vT/pT/gram for tile nt, consume vT with
        # chunk 0's U matmuls in the same iteration. E issues one nt ahead.
        # PSUM: upool(4) + epool(3: gram + rotating e_t/pt_ps) + aux(1: pv)
        # -- the same pools serve chunks 1-3, so there is no pool-scope
        # handoff (and no PE idle) at the fused->chunks transition.
        u_ps[0] = [upool.tile([P, FREE], F32, tag='u', name=f'u0_{i}')
                   for i in range(MS)]
        s_sb[0] = ssb.tile([P, FREE], BF16, tag='s', name='s0')
        if True:
            emit_e(0, 0, epool)
            emit_e(0, 1, epool)
            for nt in range(NT):
                nsl = slice(nt * P, (nt + 1) * P)
                pv = aux.tile([P, C], F32, tag='aux')
                for kcp in range(KC // 2):
                    nc.tensor.matmul(
                        pv[:], lhsT=x8_sb[:, 2 * kcp:2 * kcp + 2, nsl],
                        rhs=wv8_sb[:, 2 * kcp:2 * kcp + 2],
                        start=(kcp == 0), stop=(kcp == KC // 2 - 1),
                        perf_mode=mybir.MatmulPerfMode.DoubleRow)
                nc.vector.tensor_add(vT_sb[:, nt], in0=pv[:], in1=bvb_sb[:])

                pt_ps = epool.tile([P, DK], F32, tag='et')
                for kc in range(KC):
                    nc.tensor.matmul(pt_ps[:], lhsT=x_sb[:, kc, nsl],
                                     rhs=wd_sb[:, kc],
                                     start=(kc == 0), stop=(kc == KC - 1))
                nc.vector.tensor_add(pT_sb[:, nt], in0=pt_ps[:], in1=bdb_sb[:])

                if nt % 2 == 1 and nt + 2 < NT:
                    # adjacent even/odd E tiles pack into one PE slot
                    emit_e(0, nt + 1, epool)
                    emit_e(0, nt + 2, epool)
                if nt > 1:
                    # consume vT/exp with a 2-iteration delay: the DVE
                    # bias-add and exp semaphores fire long before U's
                    # LDWEIGHTS wants them, so the weight loads prefetch
                    emit_u(0, nt - 2)
            emit_u(0, NT - 2)
            emit_u(0, NT - 1)
            # free chunk 0's U banks right away (plain copies, no Z dep)
            o1_t[0] = []
            for ms in range(MS):
                o1 = otp.tile([P, FREE], F32, tag='o')
                nc.vector.tensor_copy(o1[:], u_ps[0][ms][:])
                o1_t[0].append(o1)


    # ================= chunks 1-3 =================
    if True:
        for mc in range(1, MCH):
            u_ps[mc] = [upool.tile([P, FREE], F32, tag='u', name=f'u{mc}_{i}')
                        for i in range(MS)]
            s_sb[mc] = ssb.tile([P, FREE], BF16, tag='s', name=f's{mc}')
            emit_e(mc, 0, epool)
            emit_e(mc, 1, epool)
            for nt in range(NT):
                if nt == 0:
                    emit_release(mc - 1, aux)
                if nt == 1 and mc > 1:
                    for ms in range(MS):
                        emit_combine(mc - 1, ms)
                if mc == MCH - 1 and nt == NT - 1:
                    # finish S and compute gamma_s/Z BEFORE the last U
                    # group: exp ran tiles ahead, so S is already complete
                    # and the Z chain overlaps U(31)'s streams instead of
                    # serializing the whole epilogue after them.
                    p_t = p_tl.pop(nt)
                    nc.vector.tensor_add(s_sb[mc][:], in0=s_sb[mc][:],
                                         in1=p_t[:])
                    z_ps = aux.tile([P, MS], F32, tag='aux', name='zlast')
                    for ms in range(MS):
                        nc.tensor.matmul(z_ps[:, ms:ms + 1],
                                         lhsT=s_sb[mc][:, ms * P:(ms + 1) * P],
                                         rhs=ones_colb[:],
                                         start=(ms == 0), stop=(ms == MS - 1))
                    zr_sb = ssb.tile([P, MS], F32, tag='zr')
                    nc.vector.reciprocal(zr_sb[:], z_ps[:])
                    zgl = ssb.tile([P, MS], F32, tag='zrg')
                    nc.vector.tensor_scalar_mul(zgl[:], in0=zr_sb[:],
                                                scalar1=gs_sb[:])
                    for ms in range(MS):
                        nc.tensor.matmul(u_ps[mc][ms][:],
                                         lhsT=p_t[:, ms * P:(ms + 1) * P],
                                         rhs=vT_sb[:, nt],
                                         start=False, stop=True)
                else:
                    emit_u(mc, nt)
                if nt % 2 == 0 and nt + 3 < NT:
                    emit_e(mc, nt + 2, epool)
                    emit_e(mc, nt + 3, epool)
                if mc == 1:
                    if nt == 1:
                        e_ps = aux.tile([DK, DK], F32, tag='aux',
                                        name='gram')
                    if 1 <= nt <= 4:
                        # channel gram, 8 tiles per iteration (the fused
                        # loop's 'et' slot now serves paired E instead)
                        for g in range(8):
                            gi = (nt - 1) * 8 + g
                            nc.tensor.matmul(e_ps[:], lhsT=pT_sb[:, gi],
                                             rhs=pT_sb[:, gi],
                                             start=(gi == 0),
                                             stop=(gi == NT - 1))
                    if nt == 4:
                        # channel softmax: c_attn = softmax(rowmax(e) - e)
                        #                         == exp(rowmin(e) - e)/rowsum
                        e_sb = const.tile([DK, DK], F32)
                        nc.vector.tensor_copy(e_sb[:], e_ps[:])
                        mn_sb = const.tile([DK, 1], F32)
                        nc.vector.tensor_reduce(mn_sb[:], e_sb[:], axis=AX.X,
                                                op=ALU.min)
                        h_sb = const.tile([DK, DK], F32)
                        nc.scalar.activation(h_sb[:], e_sb[:], ACTF.Exp,
                                             bias=mn_sb[:], scale=-1.0)
                        zc_sb = const.tile([DK, 1], F32)
                        nc.vector.tensor_reduce(zc_sb[:], h_sb[:], axis=AX.X,
                                                op=ALU.add)
                        nc.vector.reciprocal(zc_sb[:], zc_sb[:])
                        cat16_sb = const.tile([DK, DK], F16)
                        nc.vector.tensor_scalar_mul(cat16_sb[:], in0=h_sb[:],
                                                    scalar1=zc_sb[:])
                    if nt == 5:
                        catT_ps = aux.tile([DK, DK], F16, tag='aux',
                                           name='catT_ps')
                        nc.tensor.transpose(catT_ps[:], cat16_sb[:],
                                            ident16[:])
                        nc.vector.tensor_copy(catT_sb[:], catT_ps[:])
                    if 7 <= nt < 7 + MCH + MT:
                        channel_tail(nt - 7, aux)
                    if 27 <= nt < 27 + MS:
                        emit_combine(0, nt - 27)
        # final chunk: nothing needs its banks freed early, so combine
        # straight from PSUM and skip the copies (Z was computed above,
        # overlapped with the last U group)
        mcl = MCH - 1
        # odd subtiles scale on ACT so the final combines pipeline
        # across two engines instead of serializing on the DVE
        for ms in range(MS):
            o2 = otp.tile([P, FREE], F32, tag='o')
            if ms % 2 == 0:
                nc.vector.scalar_tensor_tensor(
                    out=o2[:], in0=u_ps[mcl][ms][:], scalar=zgl[:, ms:ms + 1],
                    in1=r_sb[:, mcl * MS + ms], op0=ALU.mult, op1=ALU.add)
            else:
                t = otp.tile([P, FREE], F32, tag='o')
                nc.scalar.activation(t[:], u_ps[mcl][ms][:], ACTF.Identity,
                                     scale=zgl[:, ms:ms + 1])
                nc.vector.tensor_add(o2[:], in0=t[:],
                                     in1=r_sb[:, mcl * MS + ms])
            nc.sync.dma_start(out_r[:, mcl * MS + ms], o2[:])

    aux_cm.__exit__(None, None, None)
    epool_cm.__exit__(None, None, None)
    otp_cm.__exit__(None, None, None)
    ssb_cm.__exit__(None, None, None)
    ptp_cm.__exit__(None, None, None)
    upool_cm.__exit__(None, None, None)
    const_cm.__exit__(None, None, None)


_CACHE = {}


def _get_compiled():
    if 'nc' in _CACHE:
        return _CACHE['nc']
    nc = bacc.Bacc("TRN2", num_devices=NCORES)
    io = {
        'x': nc.dram_tensor('x', [C, N], F16, kind='ExternalInput').ap(),
        'x8': nc.dram_tensor('x8', [C, N], mybir.dt.float8e4,
                             kind='ExternalInput').ap(),
        'wv8': nc.dram_tensor('wv8', [C, C], mybir.dt.float8e4,
                              kind='ExternalInput').ap(),
        'xmT': nc.dram_tensor('xmT', [M, C], F16, kind='ExternalInput').ap(),
        'consts': nc.dram_tensor('consts', [P, PKB], mybir.dt.uint8,
                                 kind='ExternalInput').ap(),
        'out': nc.dram_tensor('out', [M, C], F32, kind='ExternalOutput').ap(),
    }
    with tile.TileContext(nc) as tc:
        _build_program(tc, io)
    nc.compile()
    _CACHE['nc'] = nc
    return nc


def make_in_maps(x, Wq, bq, Wk, bk, Wv, bv, gamma_s, Wd, bd, Wu, bu, gamma_c):
    """Build the 8 per-core input dicts from the full problem inputs."""
    f32 = lambda a: np.ascontiguousarray(np.asarray(a, dtype=np.float32))
    f16 = lambda a: np.ascontiguousarray(np.asarray(a, dtype=np.float32)
                                         .astype(np.float16))
    x = f32(x).reshape(B, C, N)

    def w_chunked(wT16):  # [C, DK] f16 -> [128, KC*DK] per-partition bytes
        return np.ascontiguousarray(
            wT16.reshape(KC, P, DK).transpose(1, 0, 2).reshape(P, KC * DK))

    def w2_chunked(wT16):  # [C, DK] -> [128, KC*128] doubled [W|W]
        w2 = np.concatenate([wT16, wT16], axis=1)  # [C, 128]
        return np.ascontiguousarray(
            w2.reshape(KC, P, P).transpose(1, 0, 2).reshape(P, KC * P))

    img = np.zeros((P, PKB), np.uint8)
    img[:, OFF_WQ:OFF_WQ + 1024] = w2_chunked(f16(np.asarray(Wq).T)).view(np.uint8)
    img[:, OFF_WK:OFF_WK + 1024] = w2_chunked(f16(np.asarray(Wk).T)).view(np.uint8)
    img[:, OFF_WD:OFF_WD + 512] = w_chunked(f16(np.asarray(Wd).T)).view(np.uint8)
    wub = np.concatenate([f16(np.asarray(Wu).T), f16(bu)[None, :]], axis=0)
    img[0:DK + 1, OFF_WUB:OFF_WUB + 1024] = np.ascontiguousarray(wub).view(np.uint8)
    bq2 = np.concatenate([f32(bq), f32(bq)])
    bk2 = np.concatenate([f32(bk), f32(bk)])
    img[:, OFF_BQ:OFF_BQ + 4] = bq2[:, None].view(np.uint8)
    img[:, OFF_BK:OFF_BK + 4] = bk2[:, None].view(np.uint8)
    img[0:DK, OFF_BD:OFF_BD + 4] = f32(bd)[:, None].view(np.uint8)
    img[0:DK, OFF_GC:OFF_GC + 4] = np.broadcast_to(
        f32(gamma_c)[:, None], (DK, 1)).copy().view(np.uint8)
    img[:, OFF_GS:OFF_GS + 4] = np.broadcast_to(
        f32(gamma_s)[:, None], (P, 1)).copy().view(np.uint8)
    img[:, OFF_BDB:OFF_BDB + 256] = np.broadcast_to(
        f32(bd)[None, :], (P, DK)).copy().view(np.uint8)
    img[:, OFF_BVB:OFF_BVB + 2048] = np.broadcast_to(
        f32(bv)[None, :], (P, C)).copy().view(np.uint8)

    import ml_dtypes
    f8 = lambda a: np.ascontiguousarray(np.asarray(a, dtype=np.float32)
                                        .astype(ml_dtypes.float8_e4m3))
    shared = {
        'wv8': f8(np.asarray(Wv).T),
        'consts': img,
    }
    in_maps = []
    for core in range(NCORES):
        b, h = divmod(core, 2)
        own = slice(h * M, (h + 1) * M)
        other = slice((1 - h) * M, (2 - h) * M)
        xp = np.concatenate([x[b][:, own], x[b][:, other]], axis=1)
        in_maps.append({
            'x': f16(xp),
            'x8': f8(xp),
            'xmT': f16(x[b][:, own].T),
            **shared,
        })
    return in_maps


def assemble_out(results):
    """Stitch the 8 per-core [M, C] outputs back to [B, C, W, H]."""
    full = np.empty((B, C, N), np.float32)
    for core, res in enumerate(results):
        b, h = divmod(core, 2)
        full[b][:, h * M:(h + 1) * M] = res['out'].T
    return full.reshape(B, C, WIDTH, HEIGHT)


def kernel(**inputs):
    nc = _get_compiled()
    in_maps = make_in_maps(**inputs)
    res = bass_utils.run_bass_kernel_spmd(nc, in_maps, core_ids=list(range(NCORES)))
    return assemble_out(res.results)


# revision 27
# speedup vs baseline: 1.0072x; 1.0072x over previous
"""Dual attention (DANet-style spatial + channel attention) on 8 Trainium2
NeuronCores.

Sharding: data-parallel over batch B=4, each batch's output positions split in
half across 2 cores -> 8 identical single-core programs, no collectives. The
host permutes each core's x so its OWN m-half occupies columns [0, M): softmax
/ sums over n are permutation-invariant, so k/v/p built in permuted order are
fine, and q / residual / output always use columns [0, M).

Per-core math (x: [512, 4096] f16, m-chunk: 2048 positions):
  spatial:  q=Wq@x[:, :M]+bq; k=Wk@x+bk; E^T[n,m]=k[:,n].q[:,m]; P=exp(E^T)
            vT[n,c]=(Wv@x+bv)^T
            U^T[m,c]=sum_n P[n,m] vT[n,c]   (lhsT = P m-slices -> out has m on
            partitions, so 1/Z is a per-PARTITION scalar: no broadcast chain)
            Z[m]=sum_n P[n,m] via 4 tiny matmuls from the DVE-accumulated S
  channel:  pT[n,d]=(Wd@x+bd)^T; e=pT^T@pT; c_attn=softmax(rowmax(e)-e)
            c2=gamma_c*(c_attn@p)[:, :M]+p[:, :M]
            R^T[m,c]=Wu@c2+bu+x[:, :M]  (bu via an appended ones-row: contract
            65; residual from host-transposed xmT f16)
  out^T[m,c] = U^T*(gamma_s/Z)[m] + R^T   -> DRAM [M, C], host transposes.

Performance structure (272us v0 baseline -> ~219us at full 2.4GHz clock;
the chip's P0 power-state can downclock the PE to 2.0GHz under sustained
load, which inflates any single measurement ~19%):
  - chunk 0 of the spatial attention is FUSED into the vT/pT production
    loop: U(0, nt) consumes vT[nt-2] (two iterations behind, so the DVE
    bias-add and exp semaphores fire long before U's weight loads want
    them), and the PE rolls from projections into attention with zero
    idle -- HAM stays at 2.4GHz for the whole kernel (PE ~98% busy between
    first and last matmul).
  - vT is produced by fp8-e4m3 DoubleRow matmuls (x8/wv8 operand pairs
    [128, 2, free]): half the matmuls of the f16 conv at ~1e-3 extra error.
  - E matmuls: contract is only 64, so q/k are computed DOUBLED ([Wq|Wq]
    conv weights -> identical copies on partitions 0-63/64-127) and even/odd
    n-tiles issue to different PE row-groups via tile_position -- adjacent E
    matmuls execute concurrently (2 per 512-col slot). E issues 2-3 tiles
    ahead of its U group so ACT exp latency is hidden.
  - one permanent PSUM pool set (u:4 + e:3 + aux:1 banks) spans conv /
    fused / chunk phases: no pool-scope handoffs, no PE bubbles between
    phases. Z is 4 tiny matmuls in one bank (one accumulation group).
  - the epilogue is per-partition: U banks release via plain DVE copies (no
    Z dependency; skipped for the last chunk), combine is one
    scalar_tensor_tensor per m-subtile, output DMAs as [M, C] rows
    (2KB/partition lines); channel c2/R^T matmuls interleave into chunk 1.
  - DMA in is 8.9MB as ~10 large descriptors (each stripes 16-ways across
    the DMA engines at ~320GB/s; the ~650ns PER-DESCRIPTOR serial issue on
    the sync queue is what must stay small). The consts image ships
    gamma_s / bd-row / bv-row pre-broadcast across partitions.
"""
import sys

sys.path.insert(0, '/opt/trn_rl_repo')

import numpy as np

import concourse.bass as bass
import concourse.tile as tile
from concourse import bacc, bass_utils, mybir
from concourse.masks import make_identity

# Problem shapes (fixed by the task spec)
B, C, WIDTH, HEIGHT = 4, 512, 64, 64
N = WIDTH * HEIGHT      # 4096 spatial positions
DK = 64                 # attention inner dim (and channel-attn dim)
NCORES = 8
M = N // 2              # 2048 output positions per core
P = 128
KC = C // P             # 4 input-channel chunks
NT = N // P             # 32 key-position tiles
FREE = 512              # matmul moving free dim (one PSUM bank of fp32)
MCH = M // FREE         # 4 m-chunks per core
MS = FREE // P          # 4 m-subtiles (128 rows) per chunk
MT = M // P             # 16 m-subtiles total

F32 = mybir.dt.float32
F16 = mybir.dt.float16
BF16 = mybir.dt.bfloat16
AX = mybir.AxisListType
ALU = mybir.AluOpType
ACTF = mybir.ActivationFunctionType

# byte layout of the packed-constants image (per partition)
OFF_WQ, OFF_WK = 0, 1024     # [128, kc, 128] f16: [Wq|Wq], [Wk|Wk] doubled
OFF_WD = 2048                # [128, kc, 64] f16
OFF_BQ, OFF_BK, OFF_BD, OFF_GC = 2560, 2564, 2568, 2572
OFF_GS = 2576                # [128, 1] f32, replicated on all partitions
OFF_P2 = 2592                # consts DMA split: everything below lands first
OFF_WUB = 2592               # [65, 512] f16: rows 0-63 Wu^T, row 64 = bu
OFF_BDB = 3616               # [128, 64] f32, bd row replicated on all partitions
OFF_BVB = 3872               # [128, 512] f32, bv row replicated on all partitions
PKB = 5920


def _build_program(tc, io):
    nc = tc.nc
    x_d, xmT_d, out_d = io['x'], io['xmT'], io['out']

    const_cm = tc.tile_pool(name='const', bufs=1)
    const = const_cm.__enter__()

    # ---- persistent SBUF tensors ----
    pk_sb = const.tile([P, PKB], mybir.dt.uint8)
    # conv weights + ACT biases first: the conv pipeline waits only on
    # this piece; wub/broadcast rows land behind x chunk 0
    nc.sync.dma_start(pk_sb[:, 0:OFF_P2], io['consts'][:, 0:OFF_P2])
    wq_sb = pk_sb[:, OFF_WQ:OFF_WQ + 1024].bitcast(F16).rearrange(
        "p (kc d) -> p kc d", kc=KC)
    wk_sb = pk_sb[:, OFF_WK:OFF_WK + 1024].bitcast(F16).rearrange(
        "p (kc d) -> p kc d", kc=KC)
    wd_sb = pk_sb[:, OFF_WD:OFF_WD + 512].bitcast(F16).rearrange(
        "p (kc d) -> p kc d", kc=KC)
    wub_sb = pk_sb[0:DK + 1, OFF_WUB:OFF_WUB + 1024].bitcast(F16)
    bq_sb = pk_sb[:, OFF_BQ:OFF_BQ + 4].bitcast(F32)
    bk_sb = pk_sb[:, OFF_BK:OFF_BK + 4].bitcast(F32)
    bd_sb = pk_sb[0:DK, OFF_BD:OFF_BD + 4].bitcast(F32)
    gc_sb = pk_sb[0:DK, OFF_GC:OFF_GC + 4].bitcast(F32)
    gs_sb = pk_sb[:, OFF_GS:OFF_GS + 4].bitcast(F32)
    bdb_sb = pk_sb[:, OFF_BDB:OFF_BDB + 256].bitcast(F32)
    bvb_sb = pk_sb[:, OFF_BVB:OFF_BVB + 2048].bitcast(F32)

    ones_colb = const.tile([P, 1], BF16)   # rhs for the tiny Z matmuls
    nc.vector.memset(ones_colb[:], 1.0)
    ident16 = const.tile([DK, DK], F16)    # for the tiny c_attn transpose
    make_identity(nc, ident16[:])

    k_sb = const.tile([P, N], F16)   # keys [d, n], rows 64-127 = copy
    q_sb = const.tile([P, M], F16)   # queries,   rows 64-127 = copy
    pc_sb = const.tile([DK, M], F16)       # channel proj on the m-slice
    c2b_sb = const.tile([DK + 1, M], F16)  # c2 rows 0-63, row 64 = ones
    pT_sb = const.tile([P, NT, DK], F16)   # channel proj transposed [n, nt, d]
    vT_sb = const.tile([P, NT, C], BF16)   # values transposed, [n, nt, c]
    catT_sb = const.tile([DK, DK], F16)    # c_attn^T for the c2 matmuls
    xmT_sb = const.tile([P, MT, C], F16)   # residual x^T on the m-slice
    r_sb = const.tile([P, MT, C], F32)     # R^T = channel-out + residual

    nc.vector.memset(c2b_sb[DK:DK + 1, :], 1.0)

    wv8_sb = const.tile([P, KC, C], mybir.dt.float8e4)  # fp8 Wv^T for DR

    out_r = out_d.rearrange("(mt p) c -> p mt c", p=P)

    upool_cm = tc.tile_pool(name='upool', bufs=MS, space='PSUM')
    upool = upool_cm.__enter__()
    epool_cm = tc.tile_pool(name='epool', bufs=3, space='PSUM')
    aux_cm = tc.tile_pool(name='aux', bufs=1, space='PSUM')
    epool = aux = None  # entered after the conv pool closes
    ptp_cm = tc.tile_pool(name='pt', bufs=9)
    ptp = ptp_cm.__enter__()
    ssb_cm = tc.tile_pool(name='ssb', bufs=3)
    ssb = ssb_cm.__enter__()
    otp_cm = tc.tile_pool(name='ot', bufs=14)
    otp = otp_cm.__enter__()

    u_ps = {}     # mc -> list of 4 PSUM tiles
    s_sb = {}     # mc -> S accumulator [128, 512] bf16
    p_tl = {}     # nt -> exp tile of the CURRENT chunk
    o1_t = {}     # mc -> 4 SBUF copies of U^T awaiting the combine
    zrg = {}      # mc -> gamma_s/Z [128, 4] f32

    def emit_e(mc, nt, epool):
        # nt parity picks the PE row-group: even tiles compute on array rows
        # 0-63, odd on 64-127, so back-to-back E matmuls run CONCURRENTLY
        # (k/q carry identical copies on partitions 64-127 for this).
        msl = slice(mc * FREE, (mc + 1) * FREE)
        nsl = slice(nt * P, (nt + 1) * P)
        h = (nt & 1) * DK
        e_t = epool.tile([P, FREE], F32, tag='et')
        nc.tensor.matmul(e_t[:], lhsT=k_sb[h:h + DK, nsl],
                         rhs=q_sb[h:h + DK, msl],
                         start=True, stop=True, tile_position=(h, 0))
        p_t = ptp.tile([P, FREE], BF16, tag='p')
        nc.scalar.activation(p_t[:], e_t[:], ACTF.Exp)
        p_tl[nt] = p_t

    def emit_u(mc, nt):
        p_t = p_tl.pop(nt)
        if nt == 0:
            nc.vector.tensor_copy(s_sb[mc][:], p_t[:])
        else:
            nc.vector.tensor_add(s_sb[mc][:], in0=s_sb[mc][:], in1=p_t[:])
        for ms in range(MS):
            nc.tensor.matmul(u_ps[mc][ms][:],
                             lhsT=p_t[:, ms * P:(ms + 1) * P],
                             rhs=vT_sb[:, nt],
                             start=(nt == 0), stop=(nt == NT - 1))

    def emit_release(mc, aux):
        # Free chunk mc's U banks (plain copies, no Z dep) and compute
        # gamma_s/Z [128, 4] -- all per-partition.
        if mc not in o1_t:
            o1_t[mc] = []
            for ms in range(MS):
                o1 = otp.tile([P, FREE], F32, tag='o')
                nc.vector.tensor_copy(o1[:], u_ps[mc][ms][:])
                o1_t[mc].append(o1)
        z_ps = aux.tile([P, MS], F32, tag='aux')
        for ms in range(MS):
            nc.tensor.matmul(z_ps[:, ms:ms + 1],
                             lhsT=s_sb[mc][:, ms * P:(ms + 1) * P],
                             rhs=ones_colb[:],
                             start=(ms == 0), stop=(ms == MS - 1))
        zr_sb = ssb.tile([P, MS], F32, tag='zr')
        nc.vector.reciprocal(zr_sb[:], z_ps[:])
        zg = ssb.tile([P, MS], F32, tag='zrg')
        nc.vector.tensor_scalar_mul(zg[:], in0=zr_sb[:], scalar1=gs_sb[:])
        zrg[mc] = zg

    def emit_combine(mc, ms, eng=None):
        # out = U^T * (gamma_s/Z) + R^T, then store one [128, 512] tile.
        # All inputs are SBUF, so this can run on gpsimd when the DVE is
        # busy (chunk 1 carries the channel-attention DVE work).
        o2 = otp.tile([P, FREE], F32, tag='o')
        (eng or nc.vector).scalar_tensor_tensor(
            out=o2[:], in0=o1_t[mc][ms][:], scalar=zrg[mc][:, ms:ms + 1],
            in1=r_sb[:, mc * MS + ms], op0=ALU.mult, op1=ALU.add)
        nc.sync.dma_start(out_r[:, mc * MS + ms], o2[:])

    def channel_tail(step, aux):
        # c2 = gamma_c * (c_attn @ p)[:, :M] + pc  (4 matmuls), then
        # R^T[mt] = (c2 | ones)^T @ (Wu^T | bu) + xmT  (16 matmuls),
        # interleaved into chunk 1 one step per nt iteration.
        if step < MCH:
            j = step
            sl = slice(j * FREE, (j + 1) * FREE)
            co_ps = aux.tile([DK, FREE], F32, tag='aux')
            nc.tensor.matmul(co_ps[:], lhsT=catT_sb[:], rhs=pc_sb[:, sl],
                             start=True, stop=True)
            nc.vector.scalar_tensor_tensor(
                out=c2b_sb[0:DK, sl], in0=co_ps[:], scalar=gc_sb[:],
                in1=pc_sb[:, sl], op0=ALU.mult, op1=ALU.add)
        else:
            mt = step - MCH
            rw_ps = aux.tile([P, C], F32, tag='aux')
            nc.tensor.matmul(rw_ps[:], lhsT=c2b_sb[:, mt * P:(mt + 1) * P],
                             rhs=wub_sb[:], start=True, stop=True)
            nc.vector.tensor_add(r_sb[:, mt], in0=rw_ps[:], in1=xmT_sb[:, mt])

    # ================= phase 1 + fused chunk 0 =================
    with tc.tile_pool(name='xp', bufs=1) as xp:
        x_sb = xp.tile([P, KC, N], F16)
        x8_sb = xp.tile([P, KC, N], mybir.dt.float8e4)
        x_r = x_d.rearrange("(kc p) n -> p kc n", p=P)
        x8_r = io['x8'].rearrange("(kc p) n -> p kc n", p=P)
        # Every dma_start is striped 16-ways across the DMA engines
        # (~320GB/s data-plane); the cost that matters is the ~650ns SERIAL
        # descriptor issue on the emitting engine queue. So: few descriptors,
        # the first x chunk first (convs start on it), side inputs issued
        # from the otherwise-idle gpsimd queue.
        for kc in range(KC):   # chunk 0 lands per-kc: conv 0 starts sooner
            nc.sync.dma_start(x_sb[:, kc, 0:FREE], x_r[:, kc, 0:FREE])
        # rest of the consts (ACT biases etc.) right behind chunk 0: the
        # first conv's bias-add needs it ~1.5us after the first matmul
        nc.sync.dma_start(pk_sb[:, OFF_P2:PKB], io['consts'][:, OFF_P2:PKB])
        nc.sync.dma_start(x_sb[:, :, FREE:2 * FREE], x_r[:, :, FREE:2 * FREE])
        nc.sync.dma_start(x_sb[:, :, 2 * FREE:4 * FREE],
                          x_r[:, :, 2 * FREE:4 * FREE])
        nc.sync.dma_start(x_sb[:, :, 4 * FREE:6 * FREE],
                          x_r[:, :, 4 * FREE:6 * FREE])
        nc.sync.dma_start(x_sb[:, :, 6 * FREE:N], x_r[:, :, 6 * FREE:N])
        nc.sync.dma_start(wv8_sb[:],
                          io['wv8'].rearrange("(kc p) c -> p kc c", p=P))
        nc.sync.dma_start(x8_sb[:, :, 0:N // 2], x8_r[:, :, 0:N // 2])
        nc.sync.dma_start(x8_sb[:, :, N // 2:N], x8_r[:, :, N // 2:N])
        nc.sync.dma_start(xmT_sb[:],
                          xmT_d.rearrange("(mt p) c -> p mt c", p=P))

        # conv projections, consuming x chunks as they land:
        # q/pc over the own half (cols 0..M), k over the full N
        with tc.tile_pool(name='ps0', bufs=2, space='PSUM') as ps0:
            for j in range(8):
                sl = slice(j * FREE, (j + 1) * FREE)
                if j < MCH:
                    pq = ps0.tile([P, FREE], F32, tag='pq')
                    for kc in range(KC):
                        nc.tensor.matmul(pq[:], lhsT=wq_sb[:, kc],
                                         rhs=x_sb[:, kc, sl],
                                         start=(kc == 0), stop=(kc == KC - 1))
                    nc.scalar.activation(q_sb[:, sl], pq[:], ACTF.Identity,
                                         bias=bq_sb[:])
                    ppc = ps0.tile([DK, FREE], F32, tag='pq')
                    for kc in range(KC):
                        nc.tensor.matmul(ppc[:], lhsT=wd_sb[:, kc],
                                         rhs=x_sb[:, kc, sl],
                                         start=(kc == 0), stop=(kc == KC - 1))
                    nc.scalar.activation(pc_sb[:, sl], ppc[:], ACTF.Identity,
                                         bias=bd_sb[:])
                pk = ps0.tile([P, FREE], F32, tag='pq')
                for kc in range(KC):
                    nc.tensor.matmul(pk[:], lhsT=wk_sb[:, kc],
                                     rhs=x_sb[:, kc, sl],
                                     start=(kc == 0), stop=(kc == KC - 1))
                nc.scalar.activation(k_sb[:, sl], pk[:], ACTF.Identity,
                                     bias=bk_sb[:])
        epool = epool_cm.__enter__()
        aux = aux_cm.__enter__()

        # fused loop: produce vT/pT/gram for tile nt, consume vT with
        # chunk 0's U matmuls in the same iteration. E issues one nt ahead.
        # PSUM: upool(4) + epool(3: gram + rotating e_t/pt_ps) + aux(1: pv)
        # -- the same pools serve chunks 1-3, so there is no pool-scope
        # handoff (and no PE idle) at the fused->chunks transition.
        u_ps[0] = [upool.tile([P, FREE], F32, tag='u', name=f'u0_{i}')
                   for i in range(MS)]
        s_sb[0] = ssb.tile([P, FREE], BF16, tag='s', name='s0')
        if True:
            emit_e(0, 0, epool)
            emit_e(0, 1, epool)
            for nt in range(NT):
                nsl = slice(nt * P, (nt + 1) * P)
                pv = aux.tile([P, C], F32, tag='aux')
                for kcp in range(KC // 2):
                    nc.tensor.matmul(
                        pv[:], lhsT=x8_sb[:, 2 * kcp:2 * kcp + 2, nsl],
                        rhs=wv8_sb[:, 2 * kcp:2 * kcp + 2],
                        start=(kcp == 0), stop=(kcp == KC // 2 - 1),
                        perf_mode=mybir.MatmulPerfMode.DoubleRow)
                nc.vector.tensor_add(vT_sb[:, nt], in0=pv[:], in1=bvb_sb[:])

                pt_ps = epool.tile([P, DK], F32, tag='et')
                for kc in range(KC):
                    nc.tensor.matmul(pt_ps[:], lhsT=x_sb[:, kc, nsl],
                                     rhs=wd_sb[:, kc],
                                     start=(kc == 0), stop=(kc == KC - 1))
                nc.vector.tensor_add(pT_sb[:, nt], in0=pt_ps[:], in1=bdb_sb[:])

                if nt % 2 == 1 and nt + 2 < NT:
                    # adjacent even/odd E tiles pack into one PE slot
                    emit_e(0, nt + 1, epool)
                    emit_e(0, nt + 2, epool)
                if nt > 1:
                    # consume vT/exp with a 2-iteration delay: the DVE
                    # bias-add and exp semaphores fire long before U's
                    # LDWEIGHTS wants them, so the weight loads prefetch
                    emit_u(0, nt - 2)
            emit_u(0, NT - 2)
            emit_u(0, NT - 1)
            # free chunk 0's U banks right away (plain copies, no Z dep)
            o1_t[0] = []
            for ms in range(MS):
                o1 = otp.tile([P, FREE], F32, tag='o')
                nc.vector.tensor_copy(o1[:], u_ps[0][ms][:])
                o1_t[0].append(o1)


    # ================= chunks 1-3 =================
    if True:
        for mc in range(1, MCH):
            u_ps[mc] = [upool.tile([P, FREE], F32, tag='u', name=f'u{mc}_{i}')
                        for i in range(MS)]
            s_sb[mc] = ssb.tile([P, FREE], BF16, tag='s', name=f's{mc}')
            emit_e(mc, 0, epool)
            emit_e(mc, 1, epool)
            for nt in range(NT):
                if nt == 0:
                    emit_release(mc - 1, aux)
                if nt == 1 and mc > 1:
                    for ms in range(MS):
                        emit_combine(mc - 1, ms)
                if mc == MCH - 1 and nt == NT - 1:
                    # finish S and compute gamma_s/Z BEFORE the last U
                    # group: exp ran tiles ahead, so S is already complete
                    # and the Z chain overlaps U(31)'s streams instead of
                    # serializing the whole epilogue after them.
                    p_t = p_tl.pop(nt)
                    nc.vector.tensor_add(s_sb[mc][:], in0=s_sb[mc][:],
                                         in1=p_t[:])
                    z_ps = aux.tile([P, MS], F32, tag='aux', name='zlast')
                    for ms in range(MS):
                        nc.tensor.matmul(z_ps[:, ms:ms + 1],
                                         lhsT=s_sb[mc][:, ms * P:(ms + 1) * P],
                                         rhs=ones_colb[:],
                                         start=(ms == 0), stop=(ms == MS - 1))
                    zr_sb = ssb.tile([P, MS], F32, tag='zr')
                    nc.vector.reciprocal(zr_sb[:], z_ps[:])
                    zgl = ssb.tile([P, MS], F32, tag='zrg')
                    nc.vector.tensor_scalar_mul(zgl[:], in0=zr_sb[:],
                                                scalar1=gs_sb[:])
                    for ms in range(MS):
                        nc.tensor.matmul(u_ps[mc][ms][:],
                                         lhsT=p_t[:, ms * P:(ms + 1) * P],
                                         rhs=vT_sb[:, nt],
                                         start=False, stop=True)
                else:
                    emit_u(mc, nt)
                if nt % 2 == 0 and nt + 3 < NT:
                    emit_e(mc, nt + 2, epool)
                    emit_e(mc, nt + 3, epool)
                if mc == 1:
                    if nt == 1:
                        e_ps = aux.tile([DK, DK], F32, tag='aux',
                                        name='gram')
                    if 1 <= nt <= 4:
                        # channel gram, 8 tiles per iteration (the fused
                        # loop's 'et' slot now serves paired E instead)
                        for g in range(8):
                            gi = (nt - 1) * 8 + g
                            nc.tensor.matmul(e_ps[:], lhsT=pT_sb[:, gi],
                                             rhs=pT_sb[:, gi],
                                             start=(gi == 0),
                                             stop=(gi == NT - 1))
                    if nt == 4:
                        # channel softmax: c_attn = softmax(rowmax(e) - e)
                        #                         == exp(rowmin(e) - e)/rowsum
                        e_sb = const.tile([DK, DK], F32)
                        nc.vector.tensor_copy(e_sb[:], e_ps[:])
                        mn_sb = const.tile([DK, 1], F32)
                        nc.vector.tensor_reduce(mn_sb[:], e_sb[:], axis=AX.X,
                                                op=ALU.min)
                        h_sb = const.tile([DK, DK], F32)
                        nc.scalar.activation(h_sb[:], e_sb[:], ACTF.Exp,
                                             bias=mn_sb[:], scale=-1.0)
                        zc_sb = const.tile([DK, 1], F32)
                        nc.vector.tensor_reduce(zc_sb[:], h_sb[:], axis=AX.X,
                                                op=ALU.add)
                        nc.vector.reciprocal(zc_sb[:], zc_sb[:])
                        cat16_sb = const.tile([DK, DK], F16)
                        nc.vector.tensor_scalar_mul(cat16_sb[:], in0=h_sb[:],
                                                    scalar1=zc_sb[:])
                    if nt == 5:
                        catT_ps = aux.tile([DK, DK], F16, tag='aux',
                                           name='catT_ps')
                        nc.tensor.transpose(catT_ps[:], cat16_sb[:],
                                            ident16[:])
                        nc.vector.tensor_copy(catT_sb[:], catT_ps[:])
                    if 7 <= nt < 7 + MCH + MT:
                        channel_tail(nt - 7, aux)
                    if 27 <= nt < 27 + MS:
                        emit_combine(0, nt - 27)
        # final chunk: nothing needs its banks freed early, so combine
        # straight from PSUM and skip the copies (Z was computed above,
        # overlapped with the last U group)
        mcl = MCH - 1
        # odd subtiles scale on ACT so the final combines pipeline
        # across two engines instead of serializing on the DVE
        for ms in range(MS):
            o2 = otp.tile([P, FREE], F32, tag='o')
            if ms % 2 == 0:
                nc.vector.scalar_tensor_tensor(
                    out=o2[:], in0=u_ps[mcl][ms][:], scalar=zgl[:, ms:ms + 1],
                    in1=r_sb[:, mcl * MS + ms], op0=ALU.mult, op1=ALU.add)
            else:
                t = otp.tile([P, FREE], F32, tag='o')
                nc.scalar.activation(t[:], u_ps[mcl][ms][:], ACTF.Identity,
                                     scale=zgl[:, ms:ms + 1])
                nc.vector.tensor_add(o2[:], in0=t[:],
                                     in1=r_sb[:, mcl * MS + ms])
            nc.sync.dma_start(out_r[:, mcl * MS + ms], o2[:])

    aux_cm.__exit__(None, None, None)
    epool_cm.__exit__(None, None, None)
    otp_cm.__exit__(None, None, None)
    ssb_cm.__exit__(None, None, None)
    ptp_cm.__exit__(None, None, None)
    upool_cm.__exit__(None, None, None)
    const_cm.__exit__(None, None, None)


_CACHE = {}


def _get_compiled():
    if 'nc' in _CACHE:
        return _CACHE['nc']
    nc = bacc.Bacc("TRN2", num_devices=NCORES)
    io = {
        'x': nc.dram_tensor('x', [C, N], F16, kind='ExternalInput').ap(),
        'x8': nc.dram_tensor('x8', [C, N], mybir.dt.float8e4,
                             kind='ExternalInput').ap(),
        'wv8': nc.dram_tensor('wv8', [C, C], mybir.dt.float8e4,
                              kind='ExternalInput').ap(),
        'xmT': nc.dram_tensor('xmT', [M, C], F16, kind='ExternalInput').ap(),
        'consts': nc.dram_tensor('consts', [P, PKB], mybir.dt.uint8,
                                 kind='ExternalInput').ap(),
        'out': nc.dram_tensor('out', [M, C], F32, kind='ExternalOutput').ap(),
    }
    with tile.TileContext(nc) as tc:
        _build_program(tc, io)
    nc.compile()
    _CACHE['nc'] = nc
    return nc


def make_in_maps(x, Wq, bq, Wk, bk, Wv, bv, gamma_s, Wd, bd, Wu, bu, gamma_c):
    """Build the 8 per-core input dicts from the full problem inputs."""
    f32 = lambda a: np.ascontiguousarray(np.asarray(a, dtype=np.float32))
    f16 = lambda a: np.ascontiguousarray(np.asarray(a, dtype=np.float32)
                                         .astype(np.float16))
    x = f32(x).reshape(B, C, N)

    def w_chunked(wT16):  # [C, DK] f16 -> [128, KC*DK] per-partition bytes
        return np.ascontiguousarray(
            wT16.reshape(KC, P, DK).transpose(1, 0, 2).reshape(P, KC * DK))

    def w2_chunked(wT16):  # [C, DK] -> [128, KC*128] doubled [W|W]
        w2 = np.concatenate([wT16, wT16], axis=1)  # [C, 128]
        return np.ascontiguousarray(
            w2.reshape(KC, P, P).transpose(1, 0, 2).reshape(P, KC * P))

    img = np.zeros((P, PKB), np.uint8)
    img[:, OFF_WQ:OFF_WQ + 1024] = w2_chunked(f16(np.asarray(Wq).T)).view(np.uint8)
    img[:, OFF_WK:OFF_WK + 1024] = w2_chunked(f16(np.asarray(Wk).T)).view(np.uint8)
    img[:, OFF_WD:OFF_WD + 512] = w_chunked(f16(np.asarray(Wd).T)).view(np.uint8)
    wub = np.concatenate([f16(np.asarray(Wu).T), f16(bu)[None, :]], axis=0)
    img[0:DK + 1, OFF_WUB:OFF_WUB + 1024] = np.ascontiguousarray(wub).view(np.uint8)
    bq2 = np.concatenate([f32(bq), f32(bq)])
    bk2 = np.concatenate([f32(bk), f32(bk)])
    img[:, OFF_BQ:OFF_BQ + 4] = bq2[:, None].view(np.uint8)
    img[:, OFF_BK:OFF_BK + 4] = bk2[:, None].view(np.uint8)
    img[0:DK, OFF_BD:OFF_BD + 4] = f32(bd)[:, None].view(np.uint8)
    img[0:DK, OFF_GC:OFF_GC + 4] = np.broadcast_to(
        f32(gamma_c)[:, None], (DK, 1)).copy().view(np.uint8)
    img[:, OFF_GS:OFF_GS + 4] = np.broadcast_to(
        f32(gamma_s)[:, None], (P, 1)).copy().view(np.uint8)
    img[:, OFF_BDB:OFF_BDB + 256] = np.broadcast_to(
        f32(bd)[None, :], (P, DK)).copy().view(np.uint8)
    img[:, OFF_BVB:OFF_BVB + 2048] = np.broadcast_to(
        f32(bv)[None, :], (P, C)).copy().view(np.uint8)

    import ml_dtypes
    f8 = lambda a: np.ascontiguousarray(np.asarray(a, dtype=np.float32)
                                        .astype(ml_dtypes.float8_e4m3))
    shared = {
        'wv8': f8(np.asarray(Wv).T),
        'consts': img,
    }
    in_maps = []
    for core in range(NCORES):
        b, h = divmod(core, 2)
        own = slice(h * M, (h + 1) * M)
        other = slice((1 - h) * M, (2 - h) * M)
        xp = np.concatenate([x[b][:, own], x[b][:, other]], axis=1)
        in_maps.append({
            'x': f16(xp),
            'x8': f8(xp),
            'xmT': f16(x[b][:, own].T),
            **shared,
        })
    return in_maps


def assemble_out(results):
    """Stitch the 8 per-core [M, C] outputs back to [B, C, W, H]."""
    full = np.empty((B, C, N), np.float32)
    for core, res in enumerate(results):
        b, h = divmod(core, 2)
        full[b][:, h * M:(h + 1) * M] = res['out'].T
    return full.reshape(B, C, WIDTH, HEIGHT)


def kernel(**inputs):
    nc = _get_compiled()
    in_maps = make_in_maps(**inputs)
    res = bass_utils.run_bass_kernel_spmd(nc, in_maps, core_ids=list(range(NCORES)))
    return assemble_out(res.results)


# revision 28
# speedup vs baseline: 1.0114x; 1.0042x over previous
"""Dual attention (DANet-style spatial + channel attention) on 8 Trainium2
NeuronCores.

Sharding: data-parallel over batch B=4, each batch's output positions split in
half across 2 cores -> 8 identical single-core programs, no collectives. The
host permutes each core's x so its OWN m-half occupies columns [0, M): softmax
/ sums over n are permutation-invariant, so k/v/p built in permuted order are
fine, and q / residual / output always use columns [0, M).

Per-core math (x: [512, 4096] f16, m-chunk: 2048 positions):
  spatial:  q=Wq@x[:, :M]+bq; k=Wk@x+bk; E^T[n,m]=k[:,n].q[:,m]; P=exp(E^T)
            vT[n,c]=(Wv@x+bv)^T
            U^T[m,c]=sum_n P[n,m] vT[n,c]   (lhsT = P m-slices -> out has m on
            partitions, so 1/Z is a per-PARTITION scalar: no broadcast chain)
            Z[m]=sum_n P[n,m] via 4 tiny matmuls from the DVE-accumulated S
  channel:  pT[n,d]=(Wd@x+bd)^T; e=pT^T@pT; c_attn=softmax(rowmax(e)-e)
            c2=gamma_c*(c_attn@p)[:, :M]+p[:, :M]
            R^T[m,c]=Wu@c2+bu+x[:, :M]  (bu via an appended ones-row: contract
            65; residual from host-transposed xmT f16)
  out^T[m,c] = U^T*(gamma_s/Z)[m] + R^T   -> DRAM [M, C], host transposes.

Performance structure (272us v0 baseline -> ~219us at full 2.4GHz clock;
the chip's P0 power-state can downclock the PE to 2.0GHz under sustained
load, which inflates any single measurement ~19%):
  - chunk 0 of the spatial attention is FUSED into the vT/pT production
    loop: U(0, nt) consumes vT[nt-2] (two iterations behind, so the DVE
    bias-add and exp semaphores fire long before U's weight loads want
    them), and the PE rolls from projections into attention with zero
    idle -- HAM stays at 2.4GHz for the whole kernel (PE ~98% busy between
    first and last matmul).
  - vT is produced by fp8-e4m3 DoubleRow matmuls (x8/wv8 operand pairs
    [128, 2, free]): half the matmuls of the f16 conv at ~1e-3 extra error.
  - E matmuls: contract is only 64, so q/k are computed DOUBLED ([Wq|Wq]
    conv weights -> identical copies on partitions 0-63/64-127) and even/odd
    n-tiles issue to different PE row-groups via tile_position -- adjacent E
    matmuls execute concurrently (2 per 512-col slot). E issues 2-3 tiles
    ahead of its U group so ACT exp latency is hidden.
  - one permanent PSUM pool set (u:4 + e:3 + aux:1 banks) spans conv /
    fused / chunk phases: no pool-scope handoffs, no PE bubbles between
    phases. Z is 4 tiny matmuls in one bank (one accumulation group).
  - the epilogue is per-partition: U banks release via plain DVE copies (no
    Z dependency; skipped for the last chunk), combine is one
    scalar_tensor_tensor per m-subtile, output DMAs as [M, C] rows
    (2KB/partition lines); channel c2/R^T matmuls interleave into chunk 1.
  - DMA in is 8.9MB as ~10 large descriptors (each stripes 16-ways across
    the DMA engines at ~320GB/s; the ~650ns PER-DESCRIPTOR serial issue on
    the sync queue is what must stay small). The consts image ships
    gamma_s / bd-row / bv-row pre-broadcast across partitions.
"""
import sys

sys.path.insert(0, '/opt/trn_rl_repo')

import numpy as np

import concourse.bass as bass
import concourse.tile as tile
from concourse import bacc, bass_utils, mybir
from concourse.masks import make_identity

# Problem shapes (fixed by the task spec)
B, C, WIDTH, HEIGHT = 4, 512, 64, 64
N = WIDTH * HEIGHT      # 4096 spatial positions
DK = 64                 # attention inner dim (and channel-attn dim)
NCORES = 8
M = N // 2              # 2048 output positions per core
P = 128
KC = C // P             # 4 input-channel chunks
NT = N // P             # 32 key-position tiles
FREE = 512              # matmul moving free dim (one PSUM bank of fp32)
MCH = M // FREE         # 4 m-chunks per core
MS = FREE // P          # 4 m-subtiles (128 rows) per chunk
MT = M // P             # 16 m-subtiles total

F32 = mybir.dt.float32
F16 = mybir.dt.float16
BF16 = mybir.dt.bfloat16
AX = mybir.AxisListType
ALU = mybir.AluOpType
ACTF = mybir.ActivationFunctionType

# byte layout of the packed-constants image (per partition)
OFF_WQ, OFF_WK = 0, 1024     # [128, kc, 128] f16: [Wq|Wq], [Wk|Wk] doubled
OFF_WD = 2048                # [128, kc, 64] f16
OFF_BQ, OFF_BK, OFF_BD, OFF_GC = 2560, 2564, 2568, 2572
OFF_GS = 2576                # [128, 1] f32, replicated on all partitions
OFF_P2 = 2592                # consts DMA split: everything below lands first
OFF_WUB = 2592               # [65, 512] f16: rows 0-63 Wu^T, row 64 = bu
OFF_BDB = 3616               # [128, 64] f32, bd row replicated on all partitions
OFF_BVB = 3872               # [128, 512] f32, bv row replicated on all partitions
PKB = 5920


def _build_program(tc, io):
    nc = tc.nc
    x_d, xmT_d, out_d = io['x'], io['xmT'], io['out']

    const_cm = tc.tile_pool(name='const', bufs=1)
    const = const_cm.__enter__()

    # ---- persistent SBUF tensors ----
    pk_sb = const.tile([P, PKB], mybir.dt.uint8)
    # conv weights + ACT biases first: the conv pipeline waits only on
    # this piece; wub/broadcast rows land behind x chunk 0
    nc.sync.dma_start(pk_sb[:, 0:OFF_P2], io['consts'][:, 0:OFF_P2])
    wq_sb = pk_sb[:, OFF_WQ:OFF_WQ + 1024].bitcast(F16).rearrange(
        "p (kc d) -> p kc d", kc=KC)
    wk_sb = pk_sb[:, OFF_WK:OFF_WK + 1024].bitcast(F16).rearrange(
        "p (kc d) -> p kc d", kc=KC)
    wd_sb = pk_sb[:, OFF_WD:OFF_WD + 512].bitcast(F16).rearrange(
        "p (kc d) -> p kc d", kc=KC)
    wub_sb = pk_sb[0:DK + 1, OFF_WUB:OFF_WUB + 1024].bitcast(F16)
    bq_sb = pk_sb[:, OFF_BQ:OFF_BQ + 4].bitcast(F32)
    bk_sb = pk_sb[:, OFF_BK:OFF_BK + 4].bitcast(F32)
    bd_sb = pk_sb[0:DK, OFF_BD:OFF_BD + 4].bitcast(F32)
    gc_sb = pk_sb[0:DK, OFF_GC:OFF_GC + 4].bitcast(F32)
    gs_sb = pk_sb[:, OFF_GS:OFF_GS + 4].bitcast(F32)
    bdb_sb = pk_sb[:, OFF_BDB:OFF_BDB + 256].bitcast(F32)
    bvb_sb = pk_sb[:, OFF_BVB:OFF_BVB + 2048].bitcast(F32)

    ones_colb = const.tile([P, 1], BF16)   # rhs for the tiny Z matmuls
    nc.vector.memset(ones_colb[:], 1.0)
    ident16 = const.tile([DK, DK], F16)    # for the tiny c_attn transpose
    make_identity(nc, ident16[:])

    k_sb = const.tile([P, N], F16)   # keys [d, n], rows 64-127 = copy
    q_sb = const.tile([P, M], F16)   # queries,   rows 64-127 = copy
    pc_sb = const.tile([DK, M], F16)       # channel proj on the m-slice
    c2b_sb = const.tile([DK + 1, M], F16)  # c2 rows 0-63, row 64 = ones
    pT_sb = const.tile([P, NT, DK], F16)   # channel proj transposed [n, nt, d]
    vT_sb = const.tile([P, NT, C], BF16)   # values transposed, [n, nt, c]
    catT_sb = const.tile([DK, DK], F16)    # c_attn^T for the c2 matmuls
    xmT_sb = const.tile([P, MT, C], F16)   # residual x^T on the m-slice
    r_sb = const.tile([P, MT, C], F32)     # R^T = channel-out + residual

    nc.vector.memset(c2b_sb[DK:DK + 1, :], 1.0)

    wv8_sb = const.tile([P, KC, C], mybir.dt.float8e4)  # fp8 Wv^T for DR

    out_r = out_d.rearrange("(mt p) c -> p mt c", p=P)

    upool_cm = tc.tile_pool(name='upool', bufs=MS, space='PSUM')
    upool = upool_cm.__enter__()
    epool_cm = tc.tile_pool(name='epool', bufs=3, space='PSUM')
    aux_cm = tc.tile_pool(name='aux', bufs=1, space='PSUM')
    epool = aux = None  # entered after the conv pool closes
    ptp_cm = tc.tile_pool(name='pt', bufs=9)
    ptp = ptp_cm.__enter__()
    ssb_cm = tc.tile_pool(name='ssb', bufs=3)
    ssb = ssb_cm.__enter__()
    otp_cm = tc.tile_pool(name='ot', bufs=14)
    otp = otp_cm.__enter__()

    u_ps = {}     # mc -> list of 4 PSUM tiles
    s_sb = {}     # mc -> S accumulator [128, 512] bf16
    p_tl = {}     # nt -> exp tile of the CURRENT chunk
    o1_t = {}     # mc -> 4 SBUF copies of U^T awaiting the combine
    zrg = {}      # mc -> gamma_s/Z [128, 4] f32

    def emit_e(mc, nt, epool):
        # nt parity picks the PE row-group: even tiles compute on array rows
        # 0-63, odd on 64-127, so back-to-back E matmuls run CONCURRENTLY
        # (k/q carry identical copies on partitions 64-127 for this).
        msl = slice(mc * FREE, (mc + 1) * FREE)
        nsl = slice(nt * P, (nt + 1) * P)
        h = (nt & 1) * DK
        e_t = epool.tile([P, FREE], F32, tag='et')
        nc.tensor.matmul(e_t[:], lhsT=k_sb[h:h + DK, nsl],
                         rhs=q_sb[h:h + DK, msl],
                         start=True, stop=True, tile_position=(h, 0))
        p_t = ptp.tile([P, FREE], BF16, tag='p')
        nc.scalar.activation(p_t[:], e_t[:], ACTF.Exp)
        p_tl[nt] = p_t

    def emit_u(mc, nt):
        p_t = p_tl.pop(nt)
        if nt == 0:
            nc.vector.tensor_copy(s_sb[mc][:], p_t[:])
        else:
            nc.vector.tensor_add(s_sb[mc][:], in0=s_sb[mc][:], in1=p_t[:])
        for ms in range(MS):
            nc.tensor.matmul(u_ps[mc][ms][:],
                             lhsT=p_t[:, ms * P:(ms + 1) * P],
                             rhs=vT_sb[:, nt],
                             start=(nt == 0), stop=(nt == NT - 1))

    def emit_release(mc, aux):
        # Free chunk mc's U banks (plain copies, no Z dep) and compute
        # gamma_s/Z [128, 4] -- all per-partition.
        if mc not in o1_t:
            o1_t[mc] = []
            for ms in range(MS):
                o1 = otp.tile([P, FREE], F32, tag='o')
                nc.vector.tensor_copy(o1[:], u_ps[mc][ms][:])
                o1_t[mc].append(o1)
        z_ps = aux.tile([P, MS], F32, tag='aux')
        for ms in range(MS):
            nc.tensor.matmul(z_ps[:, ms:ms + 1],
                             lhsT=s_sb[mc][:, ms * P:(ms + 1) * P],
                             rhs=ones_colb[:],
                             start=(ms == 0), stop=(ms == MS - 1))
        zr_sb = ssb.tile([P, MS], F32, tag='zr')
        nc.vector.reciprocal(zr_sb[:], z_ps[:])
        zg = ssb.tile([P, MS], F32, tag='zrg')
        nc.vector.tensor_scalar_mul(zg[:], in0=zr_sb[:], scalar1=gs_sb[:])
        zrg[mc] = zg

    def emit_combine(mc, ms, eng=None):
        # out = U^T * (gamma_s/Z) + R^T, then store one [128, 512] tile.
        # All inputs are SBUF, so this can run on gpsimd when the DVE is
        # busy (chunk 1 carries the channel-attention DVE work).
        o2 = otp.tile([P, FREE], F32, tag='o')
        (eng or nc.vector).scalar_tensor_tensor(
            out=o2[:], in0=o1_t[mc][ms][:], scalar=zrg[mc][:, ms:ms + 1],
            in1=r_sb[:, mc * MS + ms], op0=ALU.mult, op1=ALU.add)
        nc.sync.dma_start(out_r[:, mc * MS + ms], o2[:])

    def channel_tail(step, aux):
        # c2 = gamma_c * (c_attn @ p)[:, :M] + pc  (4 matmuls), then
        # R^T[mt] = (c2 | ones)^T @ (Wu^T | bu) + xmT  (16 matmuls),
        # interleaved into chunk 1 one step per nt iteration.
        if step < MCH:
            j = step
            sl = slice(j * FREE, (j + 1) * FREE)
            co_ps = aux.tile([DK, FREE], F32, tag='aux')
            nc.tensor.matmul(co_ps[:], lhsT=catT_sb[:], rhs=pc_sb[:, sl],
                             start=True, stop=True)
            nc.vector.scalar_tensor_tensor(
                out=c2b_sb[0:DK, sl], in0=co_ps[:], scalar=gc_sb[:],
                in1=pc_sb[:, sl], op0=ALU.mult, op1=ALU.add)
        else:
            mt = step - MCH
            rw_ps = aux.tile([P, C], F32, tag='aux')
            nc.tensor.matmul(rw_ps[:], lhsT=c2b_sb[:, mt * P:(mt + 1) * P],
                             rhs=wub_sb[:], start=True, stop=True)
            nc.vector.tensor_add(r_sb[:, mt], in0=rw_ps[:], in1=xmT_sb[:, mt])

    # ================= phase 1 + fused chunk 0 =================
    with tc.tile_pool(name='xp', bufs=1) as xp:
        x_sb = xp.tile([P, KC, N], F16)
        x8_sb = xp.tile([P, KC, N], mybir.dt.float8e4)
        x_r = x_d.rearrange("(kc p) n -> p kc n", p=P)
        x8_r = io['x8'].rearrange("(kc p) n -> p kc n", p=P)
        # Every dma_start is striped 16-ways across the DMA engines
        # (~320GB/s data-plane); the cost that matters is the ~650ns SERIAL
        # descriptor issue on the emitting engine queue. So: few descriptors,
        # the first x chunk first (convs start on it), side inputs issued
        # from the otherwise-idle gpsimd queue.
        for kc in range(KC):   # chunk 0 lands per-kc: conv 0 starts sooner
            nc.sync.dma_start(x_sb[:, kc, 0:FREE], x_r[:, kc, 0:FREE])
        # rest of the consts (ACT biases etc.) right behind chunk 0: the
        # first conv's bias-add needs it ~1.5us after the first matmul
        nc.sync.dma_start(pk_sb[:, OFF_P2:PKB], io['consts'][:, OFF_P2:PKB])
        nc.sync.dma_start(x_sb[:, :, FREE:2 * FREE], x_r[:, :, FREE:2 * FREE])
        nc.sync.dma_start(x_sb[:, :, 2 * FREE:4 * FREE],
                          x_r[:, :, 2 * FREE:4 * FREE])
        nc.sync.dma_start(x_sb[:, :, 4 * FREE:6 * FREE],
                          x_r[:, :, 4 * FREE:6 * FREE])
        nc.sync.dma_start(x_sb[:, :, 6 * FREE:N], x_r[:, :, 6 * FREE:N])
        nc.sync.dma_start(wv8_sb[:],
                          io['wv8'].rearrange("(kc p) c -> p kc c", p=P))
        nc.sync.dma_start(x8_sb[:, :, 0:N // 2], x8_r[:, :, 0:N // 2])
        nc.sync.dma_start(x8_sb[:, :, N // 2:N], x8_r[:, :, N // 2:N])
        nc.sync.dma_start(xmT_sb[:],
                          xmT_d.rearrange("(mt p) c -> p mt c", p=P))

        # conv projections, consuming x chunks as they land:
        # q/pc over the own half (cols 0..M), k over the full N
        with tc.tile_pool(name='ps0', bufs=3, space='PSUM') as ps0:
            for j in range(8):
                sl = slice(j * FREE, (j + 1) * FREE)
                if j < MCH:
                    pq = ps0.tile([P, FREE], F32, tag='pq')
                    for kc in range(KC):
                        nc.tensor.matmul(pq[:], lhsT=wq_sb[:, kc],
                                         rhs=x_sb[:, kc, sl],
                                         start=(kc == 0), stop=(kc == KC - 1))
                    nc.scalar.activation(q_sb[:, sl], pq[:], ACTF.Identity,
                                         bias=bq_sb[:])
                    ppc = ps0.tile([DK, FREE], F32, tag='pq')
                    for kc in range(KC):
                        nc.tensor.matmul(ppc[:], lhsT=wd_sb[:, kc],
                                         rhs=x_sb[:, kc, sl],
                                         start=(kc == 0), stop=(kc == KC - 1))
                    nc.scalar.activation(pc_sb[:, sl], ppc[:], ACTF.Identity,
                                         bias=bd_sb[:])
                pk = ps0.tile([P, FREE], F32, tag='pq')
                for kc in range(KC):
                    nc.tensor.matmul(pk[:], lhsT=wk_sb[:, kc],
                                     rhs=x_sb[:, kc, sl],
                                     start=(kc == 0), stop=(kc == KC - 1))
                nc.scalar.activation(k_sb[:, sl], pk[:], ACTF.Identity,
                                     bias=bk_sb[:])
        epool = epool_cm.__enter__()
        aux = aux_cm.__enter__()

        # fused loop: produce vT/pT/gram for tile nt, consume vT with
        # chunk 0's U matmuls in the same iteration. E issues one nt ahead.
        # PSUM: upool(4) + epool(3: gram + rotating e_t/pt_ps) + aux(1: pv)
        # -- the same pools serve chunks 1-3, so there is no pool-scope
        # handoff (and no PE idle) at the fused->chunks transition.
        u_ps[0] = [upool.tile([P, FREE], F32, tag='u', name=f'u0_{i}')
                   for i in range(MS)]
        s_sb[0] = ssb.tile([P, FREE], BF16, tag='s', name='s0')
        if True:
            emit_e(0, 0, epool)
            emit_e(0, 1, epool)
            for nt in range(NT):
                nsl = slice(nt * P, (nt + 1) * P)
                pv = aux.tile([P, C], F32, tag='aux')
                for kcp in range(KC // 2):
                    nc.tensor.matmul(
                        pv[:], lhsT=x8_sb[:, 2 * kcp:2 * kcp + 2, nsl],
                        rhs=wv8_sb[:, 2 * kcp:2 * kcp + 2],
                        start=(kcp == 0), stop=(kcp == KC // 2 - 1),
                        perf_mode=mybir.MatmulPerfMode.DoubleRow)
                nc.vector.tensor_add(vT_sb[:, nt], in0=pv[:], in1=bvb_sb[:])

                pt_ps = epool.tile([P, DK], F32, tag='et')
                for kc in range(KC):
                    nc.tensor.matmul(pt_ps[:], lhsT=x_sb[:, kc, nsl],
                                     rhs=wd_sb[:, kc],
                                     start=(kc == 0), stop=(kc == KC - 1))
                nc.vector.tensor_add(pT_sb[:, nt], in0=pt_ps[:], in1=bdb_sb[:])

                if nt % 2 == 1 and nt + 2 < NT:
                    # adjacent even/odd E tiles pack into one PE slot
                    emit_e(0, nt + 1, epool)
                    emit_e(0, nt + 2, epool)
                if nt > 1:
                    # consume vT/exp with a 2-iteration delay: the DVE
                    # bias-add and exp semaphores fire long before U's
                    # LDWEIGHTS wants them, so the weight loads prefetch
                    emit_u(0, nt - 2)
            emit_u(0, NT - 2)
            emit_u(0, NT - 1)
            # free chunk 0's U banks right away (plain copies, no Z dep)
            o1_t[0] = []
            for ms in range(MS):
                o1 = otp.tile([P, FREE], F32, tag='o')
                nc.vector.tensor_copy(o1[:], u_ps[0][ms][:])
                o1_t[0].append(o1)


    # ================= chunks 1-3 =================
    if True:
        for mc in range(1, MCH):
            u_ps[mc] = [upool.tile([P, FREE], F32, tag='u', name=f'u{mc}_{i}')
                        for i in range(MS)]
            s_sb[mc] = ssb.tile([P, FREE], BF16, tag='s', name=f's{mc}')
            emit_e(mc, 0, epool)
            emit_e(mc, 1, epool)
            for nt in range(NT):
                if nt == 0:
                    emit_release(mc - 1, aux)
                if nt == 1 and mc > 1:
                    for ms in range(MS):
                        emit_combine(mc - 1, ms)
                if mc == MCH - 1 and nt == NT - 1:
                    # finish S and compute gamma_s/Z BEFORE the last U
                    # group: exp ran tiles ahead, so S is already complete
                    # and the Z chain overlaps U(31)'s streams instead of
                    # serializing the whole epilogue after them.
                    p_t = p_tl.pop(nt)
                    nc.vector.tensor_add(s_sb[mc][:], in0=s_sb[mc][:],
                                         in1=p_t[:])
                    z_ps = aux.tile([P, MS], F32, tag='aux', name='zlast')
                    for ms in range(MS):
                        nc.tensor.matmul(z_ps[:, ms:ms + 1],
                                         lhsT=s_sb[mc][:, ms * P:(ms + 1) * P],
                                         rhs=ones_colb[:],
                                         start=(ms == 0), stop=(ms == MS - 1))
                    zr_sb = ssb.tile([P, MS], F32, tag='zr')
                    nc.vector.reciprocal(zr_sb[:], z_ps[:])
                    zgl = ssb.tile([P, MS], F32, tag='zrg')
                    nc.vector.tensor_scalar_mul(zgl[:], in0=zr_sb[:],
                                                scalar1=gs_sb[:])
                    for ms in range(MS):
                        nc.tensor.matmul(u_ps[mc][ms][:],
                                         lhsT=p_t[:, ms * P:(ms + 1) * P],
                                         rhs=vT_sb[:, nt],
                                         start=False, stop=True)
                else:
                    emit_u(mc, nt)
                if nt % 2 == 0 and nt + 3 < NT:
                    emit_e(mc, nt + 2, epool)
                    emit_e(mc, nt + 3, epool)
                if mc == 1:
                    if nt == 1:
                        e_ps = aux.tile([DK, DK], F32, tag='aux',
                                        name='gram')
                    if 1 <= nt <= 4:
                        # channel gram, 8 tiles per iteration (the fused
                        # loop's 'et' slot now serves paired E instead)
                        for g in range(8):
                            gi = (nt - 1) * 8 + g
                            nc.tensor.matmul(e_ps[:], lhsT=pT_sb[:, gi],
                                             rhs=pT_sb[:, gi],
                                             start=(gi == 0),
                                             stop=(gi == NT - 1))
                    if nt == 4:
                        # channel softmax: c_attn = softmax(rowmax(e) - e)
                        #                         == exp(rowmin(e) - e)/rowsum
                        e_sb = const.tile([DK, DK], F32)
                        nc.vector.tensor_copy(e_sb[:], e_ps[:])
                        mn_sb = const.tile([DK, 1], F32)
                        nc.vector.tensor_reduce(mn_sb[:], e_sb[:], axis=AX.X,
                                                op=ALU.min)
                        h_sb = const.tile([DK, DK], F32)
                        nc.scalar.activation(h_sb[:], e_sb[:], ACTF.Exp,
                                             bias=mn_sb[:], scale=-1.0)
                        zc_sb = const.tile([DK, 1], F32)
                        nc.vector.tensor_reduce(zc_sb[:], h_sb[:], axis=AX.X,
                                                op=ALU.add)
                        nc.vector.reciprocal(zc_sb[:], zc_sb[:])
                        cat16_sb = const.tile([DK, DK], F16)
                        nc.vector.tensor_scalar_mul(cat16_sb[:], in0=h_sb[:],
                                                    scalar1=zc_sb[:])
                    if nt == 5:
                        catT_ps = aux.tile([DK, DK], F16, tag='aux',
                                           name='catT_ps')
                        nc.tensor.transpose(catT_ps[:], cat16_sb[:],
                                            ident16[:])
                        nc.vector.tensor_copy(catT_sb[:], catT_ps[:])
                    if 7 <= nt < 7 + MCH + MT:
                        channel_tail(nt - 7, aux)
                    if 27 <= nt < 27 + MS:
                        emit_combine(0, nt - 27)
        # final chunk: nothing needs its banks freed early, so combine
        # straight from PSUM and skip the copies (Z was computed above,
        # overlapped with the last U group)
        mcl = MCH - 1
        # odd subtiles scale on ACT so the final combines pipeline
        # across two engines instead of serializing on the DVE
        for ms in range(MS):
            o2 = otp.tile([P, FREE], F32, tag='o')
            if ms % 2 == 0:
                nc.vector.scalar_tensor_tensor(
                    out=o2[:], in0=u_ps[mcl][ms][:], scalar=zgl[:, ms:ms + 1],
                    in1=r_sb[:, mcl * MS + ms], op0=ALU.mult, op1=ALU.add)
            else:
                t = otp.tile([P, FREE], F32, tag='o')
                nc.scalar.activation(t[:], u_ps[mcl][ms][:], ACTF.Identity,
                                     scale=zgl[:, ms:ms + 1])
                nc.vector.tensor_add(o2[:], in0=t[:],
                                     in1=r_sb[:, mcl * MS + ms])
            nc.sync.dma_start(out_r[:, mcl * MS + ms], o2[:])

    aux_cm.__exit__(None, None, None)
    epool_cm.__exit__(None, None, None)
    otp_cm.__exit__(None, None, None)
    ssb_cm.__exit__(None, None, None)
    ptp_cm.__exit__(None, None, None)
    upool_cm.__exit__(None, None, None)
    const_cm.__exit__(None, None, None)


_CACHE = {}


def _get_compiled():
    if 'nc' in _CACHE:
        return _CACHE['nc']
    nc = bacc.Bacc("TRN2", num_devices=NCORES)
    io = {
        'x': nc.dram_tensor('x', [C, N], F16, kind='ExternalInput').ap(),
        'x8': nc.dram_tensor('x8', [C, N], mybir.dt.float8e4,
                             kind='ExternalInput').ap(),
        'wv8': nc.dram_tensor('wv8', [C, C], mybir.dt.float8e4,
                              kind='ExternalInput').ap(),
        'xmT': nc.dram_tensor('xmT', [M, C], F16, kind='ExternalInput').ap(),
        'consts': nc.dram_tensor('consts', [P, PKB], mybir.dt.uint8,
                                 kind='ExternalInput').ap(),
        'out': nc.dram_tensor('out', [M, C], F32, kind='ExternalOutput').ap(),
    }
    with tile.TileContext(nc) as tc:
        _build_program(tc, io)
    nc.compile()
    _CACHE['nc'] = nc
    return nc


def make_in_maps(x, Wq, bq, Wk, bk, Wv, bv, gamma_s, Wd, bd, Wu, bu, gamma_c):
    """Build the 8 per-core input dicts from the full problem inputs."""
    f32 = lambda a: np.ascontiguousarray(np.asarray(a, dtype=np.float32))
    f16 = lambda a: np.ascontiguousarray(np.asarray(a, dtype=np.float32)
                                         .astype(np.float16))
    x = f32(x).reshape(B, C, N)

    def w_chunked(wT16):  # [C, DK] f16 -> [128, KC*DK] per-partition bytes
        return np.ascontiguousarray(
            wT16.reshape(KC, P, DK).transpose(1, 0, 2).reshape(P, KC * DK))

    def w2_chunked(wT16):  # [C, DK] -> [128, KC*128] doubled [W|W]
        w2 = np.concatenate([wT16, wT16], axis=1)  # [C, 128]
        return np.ascontiguousarray(
            w2.reshape(KC, P, P).transpose(1, 0, 2).reshape(P, KC * P))

    img = np.zeros((P, PKB), np.uint8)
    img[:, OFF_WQ:OFF_WQ + 1024] = w2_chunked(f16(np.asarray(Wq).T)).view(np.uint8)
    img[:, OFF_WK:OFF_WK + 1024] = w2_chunked(f16(np.asarray(Wk).T)).view(np.uint8)
    img[:, OFF_WD:OFF_WD + 512] = w_chunked(f16(np.asarray(Wd).T)).view(np.uint8)
    wub = np.concatenate([f16(np.asarray(Wu).T), f16(bu)[None, :]], axis=0)
    img[0:DK + 1, OFF_WUB:OFF_WUB + 1024] = np.ascontiguousarray(wub).view(np.uint8)
    bq2 = np.concatenate([f32(bq), f32(bq)])
    bk2 = np.concatenate([f32(bk), f32(bk)])
    img[:, OFF_BQ:OFF_BQ + 4] = bq2[:, None].view(np.uint8)
    img[:, OFF_BK:OFF_BK + 4] = bk2[:, None].view(np.uint8)
    img[0:DK, OFF_BD:OFF_BD + 4] = f32(bd)[:, None].view(np.uint8)
    img[0:DK, OFF_GC:OFF_GC + 4] = np.broadcast_to(
        f32(gamma_c)[:, None], (DK, 1)).copy().view(np.uint8)
    img[:, OFF_GS:OFF_GS + 4] = np.broadcast_to(
        f32(gamma_s)[:, None], (P, 1)).copy().view(np.uint8)
    img[:, OFF_BDB:OFF_BDB + 256] = np.broadcast_to(
        f32(bd)[None, :], (P, DK)).copy().view(np.uint8)
    img[:, OFF_BVB:OFF_BVB + 2048] = np.broadcast_to(
        f32(bv)[None, :], (P, C)).copy().view(np.uint8)

    import ml_dtypes
    f8 = lambda a: np.ascontiguousarray(np.asarray(a, dtype=np.float32)
                                        .astype(ml_dtypes.float8_e4m3))
    shared = {
        'wv8': f8(np.asarray(Wv).T),
        'consts': img,
    }
    in_maps = []
    for core in range(NCORES):
        b, h = divmod(core, 2)
        own = slice(h * M, (h + 1) * M)
        other = slice((1 - h) * M, (2 - h) * M)
        xp = np.concatenate([x[b][:, own], x[b][:, other]], axis=1)
        in_maps.append({
            'x': f16(xp),
            'x8': f8(xp),
            'xmT': f16(x[b][:, own].T),
            **shared,
        })
    return in_maps


def assemble_out(results):
    """Stitch the 8 per-core [M, C] outputs back to [B, C, W, H]."""
    full = np.empty((B, C, N), np.float32)
    for core, res in enumerate(results):
        b, h = divmod(core, 2)
        full[b][:, h * M:(h + 1) * M] = res['out'].T
    return full.reshape(B, C, WIDTH, HEIGHT)


def kernel(**inputs):
    nc = _get_compiled()
    in_maps = make_in_maps(**inputs)
    res = bass_utils.run_bass_kernel_spmd(nc, in_maps, core_ids=list(range(NCORES)))
    return assemble_out(res.results)
